# revision 1
# baseline (speedup 1.0000x reference)
"""Chamfer distance kernel for Trainium2 (8 NeuronCores, data-parallel over batch).

Input : x, y float32 [16, 4096, 3]
Output: scalar float32 = mean_b [ mean_n min_m ||x_bn - y_bm||^2
                                + mean_m min_n ||x_bn - y_bm||^2 ]

Per core (2 batches). For each batch and each 128-row block of x points:
  s_k = Square(-yb_k + x_k)    (ScalarE; yb_k = y coord k broadcast to all
                                partitions [128,4096], x_k per-partition bias)
  d   = s_0 + s_1 + s_2        (VectorE adds)   -> d[p, m] = ||x_n - y_m||^2
  dirA: reduce_min(d) over m   -> row NN dist    (VectorE)
  dirB: colrun = min(colrun,d) running over row blocks (VectorE)
Batch finalize: colrun -> negate -> gpsimd partition_all_reduce(max) -> per-m
NN dists; reduce_sums + partition_all_reduce(add) -> scalar; accumulate.
Host: builds coordinate layouts (O(B*N)), shards batches 2-per-core, sums 8
partial sums / 16.
"""
import sys

sys.path.insert(0, "/opt/trn_rl_repo")

import numpy as np

import concourse.bacc as bacc
import concourse.bass as bass
import concourse.bass_isa as bass_isa
import concourse.tile as tile
from concourse import mybir
from concourse.alu_op_type import AluOpType
from concourse.bass_utils import run_bass_kernel_spmd

F32 = mybir.dt.float32
X = mybir.AxisListType.X
MIN = AluOpType.min
Square = mybir.ActivationFunctionType.Square

B, N, D3 = 16, 4096, 3
NCORES = 8
BPC = B // NCORES           # batches per core
RB = N // 128               # 32 row blocks


def _build_nc(repeat: int = 1):
    nc = bacc.Bacc("TRN2", target_bir_lowering=False, debug=False, num_devices=NCORES)
    # ys[b*4+k, m] = y[b, m, k] for k<3, y2[b, m] for k=3
    # xs[p, b*128 + k*32 + r] = 2*x[b, 128r+p, k] for k<3, x2[b, 128r+p] for k=3
    ys_d = nc.dram_tensor("ys", [BPC * 4, N], F32, kind="ExternalInput").ap()
    xs_d = nc.dram_tensor("xs", [128, BPC * 4 * RB], F32, kind="ExternalInput").ap()
    out_d = nc.dram_tensor("out", [1, 1], F32, kind="ExternalOutput").ap()

    with tile.TileContext(nc) as tc:
        import contextlib
        with contextlib.ExitStack() as ctx:
            const = ctx.enter_context(tc.tile_pool(name="const", bufs=1))
            acc = ctx.enter_context(tc.tile_pool(name="acc", bufs=1))
            ybp = ctx.enter_context(tc.tile_pool(name="ybp", bufs=4))
            wk = ctx.enter_context(tc.tile_pool(name="wk", bufs=1))

            xs_t = const.tile([128, BPC * 4 * RB], F32, name="xs_t")
            nc.gpsimd.dma_start(xs_t[:], xs_d[:])

            def load_yb(b):
                tiles = []
                for k in range(4):
                    t = ybp.tile([128, N], F32, name=f"yb{k}", tag="yb")
                    src = ys_d[4 * b + k : 4 * b + k + 1, :]
                    bcast = bass.AP(tensor=src.tensor, offset=src.offset,
                                    ap=[[0, 128]] + [list(p) for p in src.ap[1:]])
                    nc.gpsimd.dma_start(t[:], bcast)
                    tiles.append(t)
                return tiles

            colrun = acc.tile([128, N], F32, name="colrun")
            rowacc = acc.tile([128, RB], F32, name="rowacc")
            stot = acc.tile([1, 1], F32, name="stot")
            s_out = acc.tile([1, 1], F32, name="s_out")
            # fixed work tiles: VectorE is serial, so plain WAW reuse is free
            t0 = wk.tile([128, N], F32, name="t0")
            a = wk.tile([128, N], F32, name="a")
            c = wk.tile([128, N], F32, name="c")
            u2 = wk.tile([128, 2 * N], F32, name="u2")
            cred = wk.tile([128, N], F32, name="cred")

            A = AluOpType
            for it in range(BPC * repeat):
                b = it % BPC
                yb = load_yb(b)

                def xsc(k, r):
                    o = b * 128 + k * RB + r
                    return xs_t[:, o : o + 1]

                for r in range(0, RB, 2):
                    # u = 2x.y - x^2 - y^2 = -||x-y||^2; two row blocks per pass
                    for h in (0, 1):
                        rr = r + h
                        seg = u2[:, h * N : (h + 1) * N]
                        nc.vector.tensor_scalar_mul(t0[:], yb[0][:], xsc(0, rr))
                        nc.vector.scalar_tensor_tensor(a[:], yb[1][:], xsc(1, rr), t0[:],
                                                       op0=A.mult, op1=A.add)
                        nc.vector.scalar_tensor_tensor(c[:], yb[2][:], xsc(2, rr), a[:],
                                                       op0=A.mult, op1=A.add)
                        nc.vector.scalar_tensor_tensor(seg, c[:], xsc(3, rr), yb[3][:],
                                                       op0=A.subtract, op1=A.subtract)
                    # fold the pair, update running column max, and one 2-wide row reduce
                    nc.vector.tensor_tensor(t0[:], u2[:, 0:N], u2[:, N : 2 * N], op=A.max)
                    if r == 0:
                        nc.vector.tensor_copy(colrun[:], t0[:])
                    else:
                        nc.vector.tensor_tensor(colrun[:], colrun[:], t0[:], op=A.max)
                    nc.vector.tensor_reduce(rowacc[:, r : r + 2],
                                            u2[:].rearrange("p (h n) -> p h n", h=2),
                                            axis=X, op=A.max)

                # batch finalize (all values are -min distances)
                rs = acc.tile([128, 1], F32, name=f"rs_{it}")
                nc.vector.reduce_sum(rs[:], rowacc[:], axis=X)
                rsr = acc.tile([128, 1], F32, name=f"rsr_{it}")
                nc.gpsimd.partition_all_reduce(rsr[:], rs[:], channels=128,
                                               reduce_op=bass_isa.ReduceOp.add)
                nc.gpsimd.partition_all_reduce(cred[:], colrun[:], channels=128,
                                               reduce_op=bass_isa.ReduceOp.max)
                cs = acc.tile([1, 1], F32, name=f"cs_{it}")
                nc.vector.reduce_sum(cs[:], cred[0:1, :], axis=X)
                bt = acc.tile([1, 1], F32, name=f"bt_{it}")
                nc.vector.tensor_add(bt[:], rsr[0:1, 0:1], cs[:])
                if it == 0:
                    nc.vector.tensor_copy(stot[:], bt[:])
                else:
                    nc.vector.tensor_add(stot[:], stot[:], bt[:])

            nc.scalar.mul(s_out[:], stot[:], -1.0 / (N * repeat))
            nc.gpsimd.dma_start(out_d[:], s_out[:])
    nc.compile()
    return nc


def _build_operands(x, y):
    """x,y [B,N,3] f32 -> per-core input maps (coordinate layouts)."""
    x = np.ascontiguousarray(x, np.float32)
    y = np.ascontiguousarray(y, np.float32)
    in_maps = []
    for core in range(NCORES):
        bs = range(core * BPC, (core + 1) * BPC)
        ys_parts, xs_parts = [], []
        for b in bs:
            y2 = (y[b] * y[b]).sum(axis=1, keepdims=True)           # [N,1]
            ys_parts.append(np.concatenate([y[b], y2], axis=1).T)   # [4, N]
            xb = x[b].reshape(RB, 128, 3)                           # [r, p, k]
            x2 = (x[b] * x[b]).sum(axis=1).reshape(RB, 128, 1)      # [r, p, 1]
            aug = np.concatenate([2.0 * xb, x2], axis=2)            # [r, p, 4]
            xs_parts.append(np.transpose(aug, (1, 2, 0)).reshape(128, 4 * RB))
        ys = np.concatenate(ys_parts, axis=0)                       # [BPC*4, N]
        xs = np.concatenate(xs_parts, axis=1)                       # [128, BPC*4*RB]
        in_maps.append({"ys": np.ascontiguousarray(ys),
                        "xs": np.ascontiguousarray(xs)})
    return in_maps


_NC_CACHE = {}


def _get_nc(repeat: int = 1):
    if repeat not in _NC_CACHE:
        _NC_CACHE[repeat] = _build_nc(repeat)
    return _NC_CACHE[repeat]


def kernel(x, y):
    x = np.asarray(x, dtype=np.float32)
    y = np.asarray(y, dtype=np.float32)
    assert x.shape == (B, N, D3) and y.shape == (B, N, D3)
    in_maps = _build_operands(x, y)
    nc = _get_nc(1)
    res = run_bass_kernel_spmd(nc, in_maps, core_ids=list(range(NCORES)))
    total = sum(float(res.results[i]["out"][0, 0]) for i in range(NCORES))
    return np.float32(total / B)

